# revision 1
# baseline (speedup 1.0000x reference)
"""Pointer-network LSTM decoder kernel for Trainium2 (Bass/Tile), SPMD over 8 cores.

Problem: B=32, S=1024, H=256 LSTM decoder with attention-pointer readout.
Per step: gates = x@W_ih.T + b_ih + h@W_hh.T + b_hh; LSTM cell; scores =
einsum('bsh,bh->bs', enc, h'); probs = softmax(scores); idx = argmax;
x_next = enc[idx]. Output: probs for all 1024 steps -> [B, S, S].

v2 architecture (vs v1 baseline):
  - Data parallel over batch: 8 cores x 4 batch rows, no collectives.
  - encW = enc @ W_ih.T + bias precomputed once (transposed layout) so the
    per-step x contribution is a row gather by argmax index.
  - The 4 batch rows are split into TWO pipelined groups (A: b0-1, B: b2-3).
    Group X's serial tail (argmax -> gather -> cell) hides under the other
    group's attention matmul streaming, keeping the PE array busy (and its
    HAM clock-gate warm) continuously.
  - Per-step softmax is GONE from the loop: h is streamed to a DRAM history
    buffer, and probs = softmax(enc @ h_hist.T) are recomputed in a batched
    end-pass (one [t=128, s=1024] score block per (batch, ttile)) that is
    emitted interleaved with the loop so it fills engine idle slots.
  - Everything stays fp32: empirically the min top-2 score gap along the
    trajectory is 6.8e-5 sigma, so reduced-precision scores (bf16/f32r)
    would flip argmaxes and diverge the whole trajectory.
"""

import os
import sys
import numpy as np

sys.path.insert(0, "/opt/trn_rl_repo")

import concourse.bass as bass
import concourse.mybir as mybir
import concourse.tile as tile
from concourse import bacc
from concourse.bass_utils import run_bass_kernel_spmd

B, S, H = 32, 1024, 256
NCORES = 8
BL = B // NCORES  # batch rows per core
F32 = mybir.dt.float32
AF = mybir.ActivationFunctionType
ALU = mybir.AluOpType
AX = mybir.AxisListType

# gate order in our layout: (i, f, o, g); reference W rows are (i, f, g, o)
GATE_PERM = np.concatenate(
    [np.arange(0, 512), np.arange(768, 1024), np.arange(512, 768)]
)

_CACHE = {}


def build_nc(T=S):
    """Build + schedule + compile the per-core Bass program (T decode steps)."""
    nc = bacc.Bacc(
        "TRN2",
        target_bir_lowering=False,
        debug=False,
        num_devices=NCORES,
    )
    enc_d = nc.dram_tensor("enc", [BL, S, H], F32, kind="ExternalInput").ap()
    wiT_d = nc.dram_tensor("wiT", [128, 2048], F32, kind="ExternalInput").ap()
    whT_d = nc.dram_tensor("whT", [128, 2048], F32, kind="ExternalInput").ap()
    biasT_d = nc.dram_tensor("biasT", [128, 32], F32, kind="ExternalInput").ap()
    ident_d = nc.dram_tensor("ident", [128, 128], F32, kind="ExternalInput").ap()
    probs_d = nc.dram_tensor("probs", [BL, S, S], F32, kind="ExternalOutput").ap()
    # h history: hist[t, p, hh*4 + b] = h_t[hh*128 + p, b]
    hist_d = nc.dram_tensor("hist", [T, 128, 8], F32).ap()

    GROUPS = ((0, 1), (2, 3))  # global batch rows per group

    with tile.TileContext(nc) as tc:
        with tc.tile_pool(name="static", bufs=1) as st:
            encT = st.tile([128, BL * 2 * S], F32)   # [h_lo, (b, hh, s)]
            encWT = st.tile([128, 8 * BL * S], F32)  # [j_lo, (jc, b, s)]
            whT = st.tile([128, 2048], F32)          # [k_lo, (kc, jc*128)]
            biasT = st.tile([128, 32], F32)          # [j_lo, (jc, b)]
            cA = st.tile([128, 4], F32)              # [h_lo, (hh, bl)]
            cB = st.tile([128, 4], F32)
            hA = st.tile([128, 4], F32)
            hB = st.tile([128, 4], F32)
            h_of = {0: hA, 1: hB}
            c_of = {0: cA, 1: cB}

            nc.sync.dma_start(whT[:, :], whT_d)
            nc.sync.dma_start(biasT[:, :], biasT_d)
            nc.gpsimd.memset(cA[:, :], 0.0)
            nc.gpsimd.memset(cB[:, :], 0.0)

            # ---- precompute encT (transpose enc into [h, s] layout) ----
            with (
                tc.tile_pool(name="pre_sb", bufs=3) as pre_sb,
                tc.tile_pool(name="pre_ps", bufs=2, space="PSUM") as pre_ps,
            ):
                ident = pre_sb.tile([128, 128], F32, tag="ident")
                nc.sync.dma_start(ident[:, :], ident_d)
                for b in range(BL):
                    for stile in range(S // 128):
                        raw = pre_sb.tile([128, H], F32, tag="raw")
                        nc.sync.dma_start(
                            raw[:, :], enc_d[b, stile * 128:(stile + 1) * 128, :]
                        )
                        for hh in range(2):
                            ps = pre_ps.tile([128, 128], F32, tag="tp", bufs=2)
                            nc.tensor.transpose(
                                ps[:, :], raw[:, hh * 128:(hh + 1) * 128], ident[:, :]
                            )
                            col = (b * 2 + hh) * S + stile * 128
                            nc.vector.tensor_copy(encT[:, col:col + 128], ps[:, :])

                # ---- precompute encWT = (enc @ W_ih.T + bias).T ----
                wiT = pre_sb.tile([128, 2048], F32, tag="wiT")
                nc.sync.dma_start(wiT[:, :], wiT_d)
                for jc in range(8):
                    for b in range(BL):
                        ps = pre_ps.tile([128, 1024], F32, tag="ew", bufs=2)
                        for kc in range(2):
                            for nh in range(2):
                                nc.tensor.matmul(
                                    ps[:, nh * 512:(nh + 1) * 512],
                                    wiT[:, kc * 1024 + jc * 128:
                                        kc * 1024 + (jc + 1) * 128],
                                    encT[:, (b * 2 + kc) * S + nh * 512:
                                         (b * 2 + kc) * S + (nh + 1) * 512],
                                    start=(kc == 0), stop=(kc == 1),
                                )
                        nc.scalar.activation(
                            encWT[:, (jc * BL + b) * S:(jc * BL + b + 1) * S],
                            ps[:, :],
                            AF.Identity,
                            bias=biasT[:, jc * 4:jc * 4 + 1],
                            scale=1.0,
                        )

            # ---- main decode loop ----
            with (
                tc.tile_pool(name="g_ps", bufs=2, space="PSUM") as g_pool,
                tc.tile_pool(name="s_ps", bufs=2, space="PSUM") as s_pool,
                tc.tile_pool(name="pp_ps", bufs=2, space="PSUM") as pp_pool,
                tc.tile_pool(name="work", bufs=3) as work,
                tc.tile_pool(name="hb", bufs=2) as hb_pool,
                tc.tile_pool(name="sb", bufs=2) as sb_pool,
            ):
                biasT3 = biasT[:, :].rearrange("p (j b) -> p j b", j=8)
                encWT4 = encWT[:, :].rearrange("p (j b s) -> p j b s", j=8, b=BL)

                def hist_dst(t, g):
                    return hist_d[t].rearrange("p (hh b) -> p hh b", hh=2)[
                        :, :, 2 * g:2 * g + 2]

                def cell(g, gsb, t):
                    """LSTM cell for group g from pre-activation gsb [128,16];
                    writes c/h and streams h to hist."""
                    hX, cX = h_of[g], c_of[g]
                    nc.scalar.activation(gsb[:, 0:12], gsb[:, 0:12], AF.Sigmoid)
                    nc.scalar.activation(gsb[:, 12:16], gsb[:, 12:16], AF.Tanh)
                    ig = work.tile([128, 4], F32, tag=f"ig{g}")
                    nc.vector.tensor_mul(ig[:, :], gsb[:, 0:4], gsb[:, 12:16])
                    nc.vector.tensor_mul(cX[:, :], gsb[:, 4:8], cX[:, :])
                    nc.vector.tensor_add(cX[:, :], cX[:, :], ig[:, :])
                    tcs = work.tile([128, 4], F32, tag=f"tcs{g}")
                    nc.scalar.activation(tcs[:, :], cX[:, :], AF.Tanh)
                    nc.vector.tensor_mul(hX[:, :], gsb[:, 8:12], tcs[:, :])
                    nc.sync.dma_start(
                        hist_dst(t, g),
                        hX[:, :].rearrange("p (hh b) -> p hh b", hh=2),
                    )

                def endpass_block(k, bg, nsteps):
                    """probs[bg, 128k:128k+nsteps, :] from the h history."""
                    hblk = hb_pool.tile([128, 1024], F32, tag="hblk")
                    if nsteps < 128:
                        nc.gpsimd.memset(hblk[:, :], 0.0)
                    nc.sync.dma_start(
                        hblk[:, 0:nsteps * 8].rearrange("p (t c) -> p t c", c=8),
                        hist_d[k * 128:k * 128 + nsteps].rearrange(
                            "t p c -> p t c"),
                    )
                    hblk3 = hblk[:, :].rearrange("p (t c) -> p t c", c=8)
                    pps = []
                    for nh in range(2):
                        pp = pp_pool.tile([128, 512], F32, tag="pp")
                        for hh in range(2):
                            nc.tensor.matmul(
                                pp[:, :],
                                hblk3[:, :, hh * 4 + bg],
                                encT[:, (bg * 2 + hh) * S + nh * 512:
                                     (bg * 2 + hh) * S + (nh + 1) * 512],
                                start=(hh == 0), stop=(hh == 1),
                            )
                        pps.append(pp)
                    nmx0 = work.tile([128, 1], F32, tag="nmx0")
                    nmx1 = work.tile([128, 1], F32, tag="nmx1")
                    nc.vector.tensor_reduce(
                        nmx0[:, :], pps[0][:, :], axis=AX.X, op=ALU.max, negate=True)
                    nc.vector.tensor_reduce(
                        nmx1[:, :], pps[1][:, :], axis=AX.X, op=ALU.max, negate=True)
                    nc.vector.tensor_tensor(
                        nmx0[:, :], nmx0[:, :], nmx1[:, :], ALU.min)
                    sblk = sb_pool.tile([128, 1024], F32, tag="sblk")
                    for nh in range(2):
                        nc.scalar.activation(
                            sblk[:, nh * 512:(nh + 1) * 512], pps[nh][:, :],
                            AF.Exp, bias=nmx0[:, 0:1])
                    bsum = work.tile([128, 1], F32, tag="bsum")
                    nc.vector.tensor_reduce(
                        bsum[:, :], sblk[:, :], axis=AX.X, op=ALU.add)
                    brec = work.tile([128, 1], F32, tag="brec")
                    nc.vector.reciprocal(brec[:, :], bsum[:, :])
                    nc.vector.tensor_scalar_mul(sblk[:, :], sblk[:, :], brec[:, 0:1])
                    nc.sync.dma_start(
                        probs_d[bg, k * 128:k * 128 + nsteps, :],
                        sblk[0:nsteps, :])

                # prologue: step-0 cell from bias only (x=0, h=0)
                for g, rows in enumerate(GROUPS):
                    gsb = work.tile([128, 16], F32, tag=f"gsb{g}")
                    nc.vector.tensor_copy(
                        gsb[:, :].rearrange("p (j b) -> p j b", j=8),
                        biasT3[:, :, rows[0]:rows[0] + 2],
                    )
                    cell(g, gsb, 0)

                rowbuf = {0: None, 1: None}
                for t in range(T - 1):
                    for g, rows in enumerate(GROUPS):
                        hX = h_of[g]
                        # -- attention scores for step t --
                        sps = s_pool.tile([128, 1024], F32, tag="s")
                        for bl in range(2):
                            bg = rows[bl]
                            for nh in range(2):
                                for kc in range(2):
                                    lhs = hX[:, kc * 2 + bl:kc * 2 + bl + 1]
                                    lhs = lhs.to_broadcast((128, 32))
                                    nc.tensor.matmul(
                                        sps[32 * bl:32 * (bl + 1),
                                            nh * 512:(nh + 1) * 512],
                                        lhs,
                                        encT[:, (bg * 2 + kc) * S + nh * 512:
                                             (bg * 2 + kc) * S + (nh + 1) * 512],
                                        start=(kc == 0), stop=(kc == 1),
                                        tile_position=(0, 32 * bl),
                                    )
                        # -- argmax (only partitions 0:64 were written) --
                        maxv = work.tile([128, 8], F32, tag=f"maxv{g}")
                        nc.vector.max(maxv[0:64, :], sps[0:64, :])
                        idx = work.tile([128, 8], mybir.dt.uint32, tag=f"idx{g}")
                        nc.vector.max_index(
                            idx[0:64, :], maxv[0:64, :], sps[0:64, :])
                        # -- gather encW rows for step t+1 --
                        rb = work.tile([128, 16], F32, tag=f"row{g}")
                        rb3 = rb[:, :].rearrange("p (j b) -> p j b", j=8)
                        for bl in range(2):
                            bg = rows[bl]
                            rv = nc.values_load(
                                idx[32 * bl:32 * bl + 1, 0:1],
                                engines=[mybir.EngineType.Activation],
                                min_val=0, max_val=S - 1,
                                skip_runtime_bounds_check=True,
                            )
                            nc.scalar.copy(
                                rb3[:, :, bl:bl + 1],
                                encWT4[:, :, bg:bg + 1, bass.ds(rv, 1)],
                            )
                        rowbuf[g] = rb
                        # -- gates matmul for step t+1 --
                        gps = g_pool.tile([128, 16], F32, tag="g")
                        for jc in range(8):
                            for kc in range(2):
                                nc.tensor.matmul(
                                    gps[:, jc * 2:(jc + 1) * 2],
                                    whT[:, kc * 1024 + jc * 128:
                                        kc * 1024 + (jc + 1) * 128],
                                    hX[:, kc * 2:(kc + 1) * 2],
                                    start=(kc == 0), stop=(kc == 1),
                                )
                        # -- cell for step t+1 --
                        gsb = work.tile([128, 16], F32, tag=f"gsb{g}")
                        nc.vector.tensor_add(gsb[:, :], gps[:, :], rowbuf[g][:, :])
                        cell(g, gsb, t + 1)
                    # -- spread the probs end-pass across the loop --
                    if t >= 128 and t % 32 == 0:
                        endpass_block(t // 128 - 1, (t % 128) // 32, 128)

                # remaining end-pass blocks
                done = set()
                for t in range(T - 1):
                    if t >= 128 and t % 32 == 0:
                        done.add((t // 128 - 1, (t % 128) // 32))
                nt = T // 128 + (1 if T % 128 else 0)
                for k in range(nt):
                    for bg in range(BL):
                        if (k, bg) not in done:
                            endpass_block(k, bg, min(128, T - k * 128))

    nc.compile()
    return nc


def _host_inputs(encoder_outputs, W_ih, W_hh, b_ih, b_hh):
    """Pure layout prep (weight transposes/permutes) on host."""
    enc = np.ascontiguousarray(np.asarray(encoder_outputs, dtype=np.float32))
    W_ih = np.asarray(W_ih, dtype=np.float32)[GATE_PERM]
    W_hh = np.asarray(W_hh, dtype=np.float32)[GATE_PERM]
    bias = (np.asarray(b_ih, dtype=np.float32)
            + np.asarray(b_hh, dtype=np.float32))[GATE_PERM]

    def t_tiles(W):  # [1024, 256] -> [128, (kc 2, jc 8)*128] with W.T tiling
        out = np.empty((128, 2048), np.float32)
        WT = W.T  # [256, 1024]
        for kc in range(2):
            for jc in range(8):
                out[:, kc * 1024 + jc * 128:kc * 1024 + (jc + 1) * 128] = \
                    WT[kc * 128:(kc + 1) * 128, jc * 128:(jc + 1) * 128]
        return np.ascontiguousarray(out)

    wiT = t_tiles(W_ih)
    whT = t_tiles(W_hh)
    biasT = np.empty((128, 32), np.float32)
    for jc in range(8):
        for b in range(BL):
            biasT[:, jc * 4 + b] = bias[jc * 128:(jc + 1) * 128]
    ident = np.eye(128, dtype=np.float32)

    in_maps = []
    for c in range(NCORES):
        in_maps.append({
            "enc": enc[c * BL:(c + 1) * BL],
            "wiT": wiT,
            "whT": whT,
            "biasT": biasT,
            "ident": ident,
        })
    return in_maps


def kernel(encoder_outputs, W_ih, W_hh, b_ih, b_hh):
    key = "nc"
    if key not in _CACHE:
        _CACHE[key] = build_nc(T=S)
    nc = _CACHE[key]
    in_maps = _host_inputs(encoder_outputs, W_ih, W_hh, b_ih, b_hh)
    res = run_bass_kernel_spmd(nc, in_maps, list(range(NCORES)))
    out = np.concatenate([res.results[c]["probs"] for c in range(NCORES)], axis=0)
    return out.astype(np.float32)



# revision 2
# speedup vs baseline: 1.0348x; 1.0348x over previous
"""Pointer-network LSTM decoder kernel for Trainium2 (Bass/Tile), SPMD over 8 cores.

Problem: B=32, S=1024, H=256 LSTM decoder with attention-pointer readout.
Per step: gates = x@W_ih.T + b_ih + h@W_hh.T + b_hh; LSTM cell; scores =
einsum('bsh,bh->bs', enc, h'); probs = softmax(scores); idx = argmax;
x_next = enc[idx]. Output: probs for all 1024 steps -> [B, S, S].

v3 architecture (vs v2 fp32 baseline, ~22.8ms):
  - All matmuls in bf16 hi/lo split arithmetic (exact to ~2^-17 rel, which
    beats fp32 matmul rounding): X = X_hi + X_lo with both parts bf16.
    bf16 matmul streams at 1 cyc/col vs fp32's 4, and full-width [128,x]
    bf16 weight loads ride the fast-weight-load path (~64cyc, overlapped).
  - TRANSPOSED attention: enc tiles are the stationary weights (static in
    the PE pipeline, zero per-step maintenance); the per-step h_hi/h_lo
    pair is the 2-column moving operand. Scores land as [128 s-positions,
    schunk] in PSUM, so every reduction engages all 128 DVE lanes instead
    of 1 partition/row as in the scores-on-free-dim layout.
  - Argmax in two levels: per-partition max8/FI8 over the 8 chunk columns,
    then a [128,1]->[1,128] PE transpose of (value, packed-index) rows and
    a second max8/FI8 on partition 0. The packed index si = 128*chunk + p
    is read back with a dynamic values_load offset, so no register math.
  - Gates matmuls use W_hh hi/lo tiles x (h_hi,h_lo) 4-col moving operand;
    the hi+lo column pairs are folded with one tensor_reduce (inner-axis
    add) straight out of PSUM.
  - Softmax stays out of the loop: h streams to DRAM as bf16 hi/lo history
    and a batched end-pass recomputes probs = softmax(enc @ h_hist.T) with
    the same 3-term bf16 split (error ~1e-5 rel on probs).
  - Validated end-to-end in closed-loop simulation: argmax trajectory
    matches the fp32 jax reference exactly on this input (min top-2 score
    gap is 5.4e-5 sigma vs ~1.6e-5 sigma worst-case split error), final
    rel err 1.5e-5.
"""

import sys
import numpy as np

sys.path.insert(0, "/opt/trn_rl_repo")

import ml_dtypes
import concourse.bass as bass
import concourse.mybir as mybir
import concourse.tile as tile
from concourse import bacc
from concourse.bass_utils import run_bass_kernel_spmd

B, S, H = 32, 1024, 256
NCORES = 8
BL = B // NCORES  # batch rows per core
F32 = mybir.dt.float32
BF16 = mybir.dt.bfloat16
U32 = mybir.dt.uint32
AF = mybir.ActivationFunctionType
ALU = mybir.AluOpType
AX = mybir.AxisListType

# gate order in our layout: (i, f, o, g); reference W rows are (i, f, g, o)
GATE_PERM = np.concatenate(
    [np.arange(0, 512), np.arange(768, 1024), np.arange(512, 768)]
)

_CACHE = {}


def build_nc(T=S):
    """Build + schedule + compile the per-core Bass program (T decode steps)."""
    nc = bacc.Bacc(
        "TRN2",
        target_bir_lowering=False,
        debug=False,
        num_devices=NCORES,
    )
    enc_d = nc.dram_tensor("enc", [BL, S, H], F32, kind="ExternalInput").ap()
    whhhi_d = nc.dram_tensor("whh_hi", [128, 2048], BF16, kind="ExternalInput").ap()
    whhlo_d = nc.dram_tensor("whh_lo", [128, 2048], BF16, kind="ExternalInput").ap()
    wihhi_d = nc.dram_tensor("wih_hi", [128, 2048], BF16, kind="ExternalInput").ap()
    wihlo_d = nc.dram_tensor("wih_lo", [128, 2048], BF16, kind="ExternalInput").ap()
    biasT_d = nc.dram_tensor("biasT", [128, 32], F32, kind="ExternalInput").ap()
    ident_d = nc.dram_tensor("ident", [128, 128], F32, kind="ExternalInput").ap()
    iota_d = nc.dram_tensor("iota", [128, 1], F32, kind="ExternalInput").ap()
    probs_d = nc.dram_tensor("probs", [BL, S, S], F32, kind="ExternalOutput").ap()
    # h history (bf16 hi/lo): hist[t, p, hh*4 + b] = h_t[hh*128 + p, b]
    histhi_d = nc.dram_tensor("hist_hi", [T, 128, 8], BF16).ap()
    histlo_d = nc.dram_tensor("hist_lo", [T, 128, 8], BF16).ap()

    GROUPS = ((0, 1), (2, 3))  # global batch rows per group

    with tile.TileContext(nc) as tc:
        with tc.tile_pool(name="static", bufs=1) as st:
            # E tiles: [h-dim 128, (b, kc, sc, 128 s)] bf16 hi/lo
            Ehi = st.tile([128, BL * 2048], BF16)
            Elo = st.tile([128, BL * 2048], BF16)
            encWT = st.tile([128, 8 * BL * S], F32)  # [j_lo, (jc, b, s)]
            whhhi = st.tile([128, 2048], BF16)       # [k_lo, (kc, jc*128)]
            whhlo = st.tile([128, 2048], BF16)
            biasT = st.tile([128, 32], F32)          # [j_lo, (jc, b)]
            ident = st.tile([128, 128], F32)
            iota = st.tile([128, 1], F32)
            cA = st.tile([128, 4], F32)              # [h_lo, (kc, bl)]
            cB = st.tile([128, 4], F32)
            hA = st.tile([128, 4], F32)
            hB = st.tile([128, 4], F32)
            hpA = st.tile([128, 8], BF16)            # [h_lo, (kc, bl, hi/lo)]
            hpB = st.tile([128, 8], BF16)
            h_of = {0: hA, 1: hB}
            c_of = {0: cA, 1: cB}
            hp_of = {0: hpA, 1: hpB}

            nc.sync.dma_start(whhhi[:, :], whhhi_d)
            nc.sync.dma_start(whhlo[:, :], whhlo_d)
            nc.sync.dma_start(biasT[:, :], biasT_d)
            nc.sync.dma_start(ident[:, :], ident_d)
            nc.sync.dma_start(iota[:, :], iota_d)
            nc.gpsimd.memset(cA[:, :], 0.0)
            nc.gpsimd.memset(cB[:, :], 0.0)

            Ehi4 = Ehi[:, :].rearrange("p (b k c s) -> p b k c s", b=BL, k=2, c=8)
            Elo4 = Elo[:, :].rearrange("p (b k c s) -> p b k c s", b=BL, k=2, c=8)

            # ---- prologue: transpose enc + split into bf16 hi/lo ----
            with (
                tc.tile_pool(name="pre_sb", bufs=3) as pre_sb,
                tc.tile_pool(name="pre_ps", bufs=2, space="PSUM") as pre_ps,
            ):
                for b in range(BL):
                    for sc in range(8):
                        raw = pre_sb.tile([128, H], F32, tag="raw")
                        nc.sync.dma_start(
                            raw[:, :], enc_d[b, sc * 128:(sc + 1) * 128, :]
                        )
                        for kc in range(2):
                            ps = pre_ps.tile([128, 128], F32, tag="tp", bufs=2)
                            nc.tensor.transpose(
                                ps[:, :], raw[:, kc * 128:(kc + 1) * 128],
                                ident[:, :]
                            )
                            # hi = bf16(x); lo = bf16(x - hi)
                            nc.scalar.copy(Ehi4[:, b, kc, sc, :], ps[:, :])
                            lo32 = pre_sb.tile([128, 128], F32, tag="lo32")
                            nc.vector.tensor_tensor(
                                lo32[:, :], ps[:, :], Ehi4[:, b, kc, sc, :],
                                ALU.subtract)
                            nc.vector.tensor_copy(Elo4[:, b, kc, sc, :],
                                                  lo32[:, :])

                # ---- encWT = (enc @ W_ih.T + bias).T via 3-term bf16 ----
                wihhi = pre_sb.tile([128, 2048], BF16, tag="wihhi")
                wihlo = pre_sb.tile([128, 2048], BF16, tag="wihlo")
                nc.sync.dma_start(wihhi[:, :], wihhi_d)
                nc.sync.dma_start(wihlo[:, :], wihlo_d)
                for jc in range(8):
                    for b in range(BL):
                        ps = pre_ps.tile([128, 1024], F32, tag="ew", bufs=2)
                        for nh in range(2):
                            first = True
                            for kc in range(2):
                                whi = wihhi[:, kc * 1024 + jc * 128:
                                            kc * 1024 + (jc + 1) * 128]
                                wlo = wihlo[:, kc * 1024 + jc * 128:
                                            kc * 1024 + (jc + 1) * 128]
                                mhi = Ehi4[:, b, kc, nh * 4:(nh + 1) * 4, :]
                                mlo = Elo4[:, b, kc, nh * 4:(nh + 1) * 4, :]
                                dst = ps[:, nh * 512:(nh + 1) * 512]
                                nc.tensor.matmul(dst, whi, mhi,
                                                 start=first, stop=False)
                                first = False
                                nc.tensor.matmul(dst, wlo, mhi,
                                                 start=False, stop=False)
                                nc.tensor.matmul(dst, whi, mlo,
                                                 start=False,
                                                 stop=(kc == 1))
                        nc.scalar.activation(
                            encWT[:, (jc * BL + b) * S:(jc * BL + b + 1) * S],
                            ps[:, :],
                            AF.Identity,
                            bias=biasT[:, jc * 4:jc * 4 + 1],
                            scale=1.0,
                        )

            # ---- main decode loop ----
            with (
                tc.tile_pool(name="sp_ps", bufs=2, space="PSUM") as sp_pool,
                tc.tile_pool(name="tp_ps", bufs=2, space="PSUM") as tp_pool,
                tc.tile_pool(name="g_ps", bufs=2, space="PSUM") as g_pool,
                tc.tile_pool(name="pp_ps", bufs=2, space="PSUM") as pp_pool,
                tc.tile_pool(name="work", bufs=3) as work,
                tc.tile_pool(name="hb", bufs=2) as hb_pool,
                tc.tile_pool(name="sb", bufs=2) as sb_pool,
            ):
                biasT3 = biasT[:, :].rearrange("p (j b) -> p j b", j=8)
                encWT4 = encWT[:, :].rearrange("p (j b s) -> p j b s", j=8, b=BL)

                def hist_dst(d, t, g):
                    return d[t].rearrange("p (hh b) -> p hh b", hh=2)[
                        :, :, 2 * g:2 * g + 2]

                def hsplit(g, t):
                    """Update hp (bf16 hi/lo of h) and stream to hist."""
                    hX, hp = h_of[g], hp_of[g]
                    hp3 = hp[:, :].rearrange("p (k t) -> p k t", k=4)
                    nc.vector.tensor_copy(hp3[:, :, 0], hX[:, :])  # hi
                    t32 = work.tile([128, 4], F32, tag=f"t32{g}")
                    nc.vector.tensor_tensor(t32[:, :], hX[:, :], hp3[:, :, 0],
                                            ALU.subtract)
                    nc.vector.tensor_copy(hp3[:, :, 1], t32[:, :])  # lo
                    nc.sync.dma_start(
                        hist_dst(histhi_d, t, g),
                        hp3[:, :, 0].rearrange("p (hh b) -> p hh b", hh=2),
                    )
                    nc.sync.dma_start(
                        hist_dst(histlo_d, t, g),
                        hp3[:, :, 1].rearrange("p (hh b) -> p hh b", hh=2),
                    )

                def cell(g, gsb, t):
                    """LSTM cell for group g from pre-activation gsb [128,16]."""
                    hX, cX = h_of[g], c_of[g]
                    nc.scalar.activation(gsb[:, 0:12], gsb[:, 0:12], AF.Sigmoid)
                    nc.scalar.activation(gsb[:, 12:16], gsb[:, 12:16], AF.Tanh)
                    ig = work.tile([128, 4], F32, tag=f"ig{g}")
                    nc.vector.tensor_mul(ig[:, :], gsb[:, 0:4], gsb[:, 12:16])
                    nc.vector.tensor_mul(cX[:, :], gsb[:, 4:8], cX[:, :])
                    nc.vector.tensor_add(cX[:, :], cX[:, :], ig[:, :])
                    tcs = work.tile([128, 4], F32, tag=f"tcs{g}")
                    nc.scalar.activation(tcs[:, :], cX[:, :], AF.Tanh)
                    nc.vector.tensor_mul(hX[:, :], gsb[:, 8:12], tcs[:, :])
                    hsplit(g, t)

                def endpass_block(k, bg, nsteps):
                    """probs[bg, 128k:128k+nsteps, :] from the h history."""
                    hbhi = hb_pool.tile([128, 1024], BF16, tag="hbhi")
                    hblo = hb_pool.tile([128, 1024], BF16, tag="hblo")
                    if nsteps < 128:
                        nc.gpsimd.memset(hbhi[:, :], 0.0)
                        nc.gpsimd.memset(hblo[:, :], 0.0)
                    nc.sync.dma_start(
                        hbhi[:, 0:nsteps * 8].rearrange("p (t c) -> p t c", c=8),
                        histhi_d[k * 128:k * 128 + nsteps].rearrange(
                            "t p c -> p t c"),
                    )
                    nc.sync.dma_start(
                        hblo[:, 0:nsteps * 8].rearrange("p (t c) -> p t c", c=8),
                        histlo_d[k * 128:k * 128 + nsteps].rearrange(
                            "t p c -> p t c"),
                    )
                    hbhi3 = hbhi[:, :].rearrange("p (t c) -> p t c", c=8)
                    hblo3 = hblo[:, :].rearrange("p (t c) -> p t c", c=8)
                    pps = []
                    for nh in range(2):
                        pp = pp_pool.tile([128, 512], F32, tag="pp")
                        first = True
                        for hh in range(2):
                            whi = hbhi3[:, :, hh * 4 + bg]
                            wlo = hblo3[:, :, hh * 4 + bg]
                            mhi = Ehi4[:, bg, hh, nh * 4:(nh + 1) * 4, :]
                            mlo = Elo4[:, bg, hh, nh * 4:(nh + 1) * 4, :]
                            nc.tensor.matmul(pp[:, :], whi, mhi,
                                             start=first, stop=False)
                            first = False
                            nc.tensor.matmul(pp[:, :], wlo, mhi,
                                             start=False, stop=False)
                            nc.tensor.matmul(pp[:, :], whi, mlo,
                                             start=False, stop=(hh == 1))
                        pps.append(pp)
                    nmx0 = work.tile([128, 1], F32, tag="nmx0")
                    nmx1 = work.tile([128, 1], F32, tag="nmx1")
                    nc.vector.tensor_reduce(
                        nmx0[:, :], pps[0][:, :], axis=AX.X, op=ALU.max,
                        negate=True)
                    nc.vector.tensor_reduce(
                        nmx1[:, :], pps[1][:, :], axis=AX.X, op=ALU.max,
                        negate=True)
                    nc.vector.tensor_tensor(
                        nmx0[:, :], nmx0[:, :], nmx1[:, :], ALU.min)
                    sblk = sb_pool.tile([128, 1024], F32, tag="sblk")
                    for nh in range(2):
                        nc.scalar.activation(
                            sblk[:, nh * 512:(nh + 1) * 512], pps[nh][:, :],
                            AF.Exp, bias=nmx0[:, 0:1])
                    bsum = work.tile([128, 1], F32, tag="bsum")
                    nc.vector.tensor_reduce(
                        bsum[:, :], sblk[:, :], axis=AX.X, op=ALU.add)
                    brec = work.tile([128, 1], F32, tag="brec")
                    nc.vector.reciprocal(brec[:, :], bsum[:, :])
                    nc.vector.tensor_scalar_mul(sblk[:, :], sblk[:, :],
                                                brec[:, 0:1])
                    nc.sync.dma_start(
                        probs_d[bg, k * 128:k * 128 + nsteps, :],
                        sblk[0:nsteps, :])

                # prologue: step-0 cell from bias only (x=0, h=0)
                for g, rows in enumerate(GROUPS):
                    gsb = work.tile([128, 16], F32, tag=f"gsb{g}")
                    nc.vector.tensor_copy(
                        gsb[:, :].rearrange("p (j b) -> p j b", j=8),
                        biasT3[:, :, rows[0]:rows[0] + 2],
                    )
                    cell(g, gsb, 0)

                for t in range(T - 1):
                    for g, rows in enumerate(GROUPS):
                        hp = hp_of[g]
                        # -- attention scores for step t (transposed layout) --
                        sp = sp_pool.tile([128, 32], F32, tag=f"sp{g}",
                                          bufs=1, name=f"sp{g}")
                        for bl in range(2):
                            bg = rows[bl]
                            mv = hp[:, :]
                            for sc in range(8):
                                dst = sp[:, (bl * 8 + sc) * 2:
                                         (bl * 8 + sc) * 2 + 2]
                                for kc in range(2):
                                    nc.tensor.matmul(
                                        dst, Ehi4[:, bg, kc, sc, :],
                                        mv[:, kc * 4 + bl * 2:
                                           kc * 4 + bl * 2 + 2],
                                        start=(kc == 0), stop=False)
                                    nc.tensor.matmul(
                                        dst, Elo4[:, bg, kc, sc, :],
                                        mv[:, kc * 4 + bl * 2:
                                           kc * 4 + bl * 2 + 2],
                                        start=False, stop=(kc == 1))
                        # -- fold hi+lo columns: [128,(16,2)] -> [128,16] --
                        sc2 = work.tile([128, 16], F32, tag=f"sc2{g}")
                        nc.vector.tensor_reduce(
                            sc2[:, :].rearrange("p (c o) -> p c o", o=1),
                            sp[:, :].rearrange("p (c t) -> p c t", t=2),
                            axis=AX.X, op=ALU.add)
                        # -- two-level argmax + gather per row --
                        tpp = tp_pool.tile([128, 512], F32, tag="tpp")
                        rb = work.tile([128, 16], F32, tag=f"row{g}")
                        rb3 = rb[:, :].rearrange("p (j b) -> p j b", j=8)
                        for bl in range(2):
                            bg = rows[bl]
                            mx1 = work.tile([128, 8], F32, tag=f"mx1{g}{bl}")
                            ix1 = work.tile([128, 8], U32, tag=f"ix1{g}{bl}")
                            nc.vector.max(mx1[:, :],
                                          sc2[:, bl * 8:(bl + 1) * 8])
                            nc.vector.max_index(ix1[:, :], mx1[:, :],
                                                sc2[:, bl * 8:(bl + 1) * 8])
                            ixf = work.tile([128, 1], F32, tag=f"ixf{g}{bl}")
                            nc.vector.tensor_copy(ixf[:, :], ix1[:, 0:1])
                            sif = work.tile([128, 1], F32, tag=f"sif{g}{bl}")
                            nc.vector.tensor_scalar(
                                sif[:, :], ixf[:, :], 128.0, iota[:, 0:1],
                                ALU.mult, ALU.add)
                            nc.tensor.transpose(
                                tpp[0:1, bl * 256:bl * 256 + 128],
                                mx1[:, 0:1], ident[:, :])
                            nc.tensor.transpose(
                                tpp[0:1, bl * 256 + 128:bl * 256 + 256],
                                sif[:, :], ident[:, :])
                            mx2 = work.tile([128, 8], F32, tag=f"mx2{g}{bl}")
                            ix2 = work.tile([128, 8], U32, tag=f"ix2{g}{bl}")
                            nc.vector.max(mx2[0:1, :],
                                          tpp[0:1, bl * 256:bl * 256 + 128])
                            nc.vector.max_index(
                                ix2[0:1, :], mx2[0:1, :],
                                tpp[0:1, bl * 256:bl * 256 + 128])
                            rvp = nc.values_load(
                                ix2[0:1, 0:1],
                                engines=[mybir.EngineType.Activation],
                                min_val=0, max_val=127,
                                skip_runtime_bounds_check=True,
                            )
                            su = work.tile([128, 1], U32, tag=f"su{g}{bl}")
                            nc.scalar.copy(
                                su[0:1, 0:1],
                                tpp[0:1, bl * 256 + 128:bl * 256 + 256][
                                    0:1, bass.ds(rvp, 1)])
                            rvs = nc.values_load(
                                su[0:1, 0:1],
                                engines=[mybir.EngineType.Activation],
                                min_val=0, max_val=S - 1,
                                skip_runtime_bounds_check=True,
                            )
                            nc.scalar.copy(
                                rb3[:, :, bl:bl + 1],
                                encWT4[:, :, bg:bg + 1, bass.ds(rvs, 1)],
                            )
                        # -- gates matmul for step t+1 --
                        gps = g_pool.tile([128, 32], F32, tag="g")
                        for jc in range(8):
                            for kc in range(2):
                                dst = gps[:, jc * 4:(jc + 1) * 4]
                                nc.tensor.matmul(
                                    dst,
                                    whhhi[:, kc * 1024 + jc * 128:
                                          kc * 1024 + (jc + 1) * 128],
                                    hp[:, kc * 4:(kc + 1) * 4],
                                    start=(kc == 0), stop=False)
                                nc.tensor.matmul(
                                    dst,
                                    whhlo[:, kc * 1024 + jc * 128:
                                          kc * 1024 + (jc + 1) * 128],
                                    hp[:, kc * 4:(kc + 1) * 4],
                                    start=False, stop=(kc == 1))
                        # -- fold gates hi+lo and add gathered x-part --
                        gsb = work.tile([128, 16], F32, tag=f"gsb{g}")
                        nc.vector.tensor_reduce(
                            gsb[:, :].rearrange("p (c o) -> p c o", o=1),
                            gps[:, :].rearrange("p (c t) -> p c t", t=2),
                            axis=AX.X, op=ALU.add)
                        nc.vector.tensor_add(gsb[:, :], gsb[:, :], rb[:, :])
                        cell(g, gsb, t + 1)
                    # -- spread the probs end-pass across the loop --
                    if t >= 128 and t % 32 == 0:
                        endpass_block(t // 128 - 1, (t % 128) // 32, 128)

                # remaining end-pass blocks
                done = set()
                for t in range(T - 1):
                    if t >= 128 and t % 32 == 0:
                        done.add((t // 128 - 1, (t % 128) // 32))
                nt = T // 128 + (1 if T % 128 else 0)
                for k in range(nt):
                    for bg in range(BL):
                        if (k, bg) not in done:
                            endpass_block(k, bg, min(128, T - k * 128))

    nc.compile()
    return nc


def _host_inputs(encoder_outputs, W_ih, W_hh, b_ih, b_hh):
    """Pure layout prep (weight transposes/permutes/splits) on host."""
    enc = np.ascontiguousarray(np.asarray(encoder_outputs, dtype=np.float32))
    W_ih = np.asarray(W_ih, dtype=np.float32)[GATE_PERM]
    W_hh = np.asarray(W_hh, dtype=np.float32)[GATE_PERM]
    bias = (np.asarray(b_ih, dtype=np.float32)
            + np.asarray(b_hh, dtype=np.float32))[GATE_PERM]

    def t_tiles(W):  # [1024, 256] -> [128, (kc 2, jc 8)*128] with W.T tiling
        out = np.empty((128, 2048), np.float32)
        WT = W.T  # [256, 1024]
        for kc in range(2):
            for jc in range(8):
                out[:, kc * 1024 + jc * 128:kc * 1024 + (jc + 1) * 128] = \
                    WT[kc * 128:(kc + 1) * 128, jc * 128:(jc + 1) * 128]
        return np.ascontiguousarray(out)

    def split(x):
        hi = x.astype(ml_dtypes.bfloat16)
        lo = (x - hi.astype(np.float32)).astype(ml_dtypes.bfloat16)
        return hi, lo

    whh_hi, whh_lo = split(t_tiles(W_hh))
    wih_hi, wih_lo = split(t_tiles(W_ih))
    biasT = np.empty((128, 32), np.float32)
    for jc in range(8):
        for b in range(BL):
            biasT[:, jc * 4 + b] = bias[jc * 128:(jc + 1) * 128]
    ident = np.eye(128, dtype=np.float32)
    iota = np.arange(128, dtype=np.float32).reshape(128, 1)

    in_maps = []
    for c in range(NCORES):
        in_maps.append({
            "enc": enc[c * BL:(c + 1) * BL],
            "whh_hi": whh_hi,
            "whh_lo": whh_lo,
            "wih_hi": wih_hi,
            "wih_lo": wih_lo,
            "biasT": biasT,
            "ident": ident,
            "iota": iota,
        })
    return in_maps


def kernel(encoder_outputs, W_ih, W_hh, b_ih, b_hh):
    key = "nc"
    if key not in _CACHE:
        _CACHE[key] = build_nc(T=S)
    nc = _CACHE[key]
    in_maps = _host_inputs(encoder_outputs, W_ih, W_hh, b_ih, b_hh)
    res = run_bass_kernel_spmd(nc, in_maps, list(range(NCORES)))
    out = np.concatenate([res.results[c]["probs"] for c in range(NCORES)], axis=0)
    return out.astype(np.float32)
